# revision 14
# baseline (speedup 1.0000x reference)
"""Trainium2 Bass kernel for single-head attention with softmax over the query axis.

Reference computation (B=4, S=4096, DIM=768, D=96):
    q = x @ Wq + bq; k = x @ Wk + bk; v = x @ Wv + bv        # [B,S,D]
    att = einsum('bqd,bkd->bqk', q, k) / sqrt(D)             # [B,Sq,Sk]
    p   = softmax(att, axis=1)                               # over the QUERY axis
    out = einsum('bqk,bkd->bqd', p, v)

Sharding: 8 cores = 4 batches x 2 key-halves. Softmax over q is local to a
key-shard (it normalizes each key-column over all queries), and the output
contraction over k is a sum over the two key-halves, done host-side.

SPMD uniformity trick: every core runs the identical program "K/V come from
rows 0:2048 of my x, Q from all 4096 rows". The host hands core (b, kh=1) a
row-rolled copy of x[b] so its key half lands in rows 0:2048; softmax over q
is permutation-invariant, and the host un-rolls that core's partial output.

Host precomputation (legal data prep inside kernel()): x is rolled,
transposed to xT [768, 4096] and cast to fp16; Wq/bq are pre-scaled by
1/sqrt(D) so no separate score scaling is needed; weights pre-cast to fp16.

The scalar (ACT) engine is the kernel's critical resource: exp over
[2048 keys x 4096 q] streams at ~1.2GHz with ~350ns of pipelined fixed
cost per [128,1024] ACTIVATE+accum chunk (~1.2us/chunk effective). The
design keeps ACT saturated from ~16us to the end:

  S-A (q 0:2048, all kk): scores in a 3-deep [128,1024] PSUM rotation
      (6 banks) so the PE runs up to two chunks ahead of ACT; the 2 spare
      banks hold projection-block accumulators — one projection/transpose
      work unit is interleaved per chunk, ordered by DMA arrival. exp
      results stay resident (expT_A) for the O2 pass.
  S-B (q 2048:4096): scores 1024-double-buffered (4 banks) + PV-hi
      accumulation (4 banks), software-pipelined one kk behind, as the
      PSUM budget dictates.
  O2: PV for q 0:2048 from expT_A.

PE is warmed with dummy matmuls on on-chip constants during the initial xT
DMA flight so the HAM clock gate is released before real work arrives
(identity/zero constants are built BEFORE the weight DMAs so the gpsimd
queue doesn't delay them), and all PSUM drains (bias adds, copies) run on
the vector engine so the scalar engine does exp exclusively.
"""

import os
import sys

import numpy as np

for _p in ("/opt/trn_rl_repo",):
    if _p not in sys.path and os.path.isdir(_p):
        sys.path.insert(0, _p)

B, S, DIM, D = 4, 4096, 768, 96
SK = S // 2          # local keys per core
N_CORES = 8
NDC = DIM // 128     # 6 dim chunks
NKK = SK // 128      # 16 local key chunks

_CACHE = {}


def _build_module():
    import concourse.bass as bass
    import concourse.tile as tile
    from concourse import bacc, mybir
    from concourse.masks import make_identity
    from concourse.tile import add_dep_helper

    fp32 = mybir.dt.float32
    fp16 = mybir.dt.float16

    nc = bacc.Bacc("TRN2", target_bir_lowering=False, debug=False,
                   num_devices=N_CORES)

    xT_ap = nc.dram_tensor("xT", [DIM, S], fp16, kind="ExternalInput").ap()
    wq_ap = nc.dram_tensor("wq", [DIM, D], fp16, kind="ExternalInput").ap()
    wk_ap = nc.dram_tensor("wk", [DIM, D], fp16, kind="ExternalInput").ap()
    wv_ap = nc.dram_tensor("wv", [DIM, D], fp16, kind="ExternalInput").ap()
    bq_ap = nc.dram_tensor("bq", [D, 1], fp32, kind="ExternalInput").ap()
    bk_ap = nc.dram_tensor("bk", [D, 1], fp32, kind="ExternalInput").ap()
    bv_ap = nc.dram_tensor("bv", [D, 1], fp32, kind="ExternalInput").ap()
    outT_ap = nc.dram_tensor("outT", [D, S], fp16, kind="ExternalOutput").ap()

    with tile.TileContext(nc) as tc:
        with (
            tc.tile_pool(name="singles", bufs=1) as singles,
            tc.tile_pool(name="acts", bufs=1) as acts,
            tc.tile_pool(name="outp", bufs=4) as outp,
        ):
            # On-chip constants FIRST: the gpsimd queue spends ~5us issuing
            # the weight-DMA descriptors, and make_identity runs on gpsimd
            # — if the DMAs go first, identity (and with it the PE warm-up
            # matmuls) is delayed to ~13us.
            identity = singles.tile([128, 128], fp16)
            make_identity(nc, identity[:])
            zeros512 = singles.tile([128, 512], fp16, tag="zeros512")
            nc.vector.memset(zeros512[:], 0.0)
            # Dummy exp: walrus places the ~2.7us ACT_TABLE_LOAD before the
            # first ACTIVATE, so trigger it while the scalar engine is
            # otherwise idle instead of on the critical path of the first
            # real exp.
            warm_in = singles.tile([128, 8], fp32, tag="warm_in")
            nc.vector.memset(warm_in[:], 0.0)
            warm_out = singles.tile([128, 8], fp32, tag="warm_out")
            nc.scalar.activation(warm_out[:], warm_in[:],
                                 mybir.ActivationFunctionType.Exp)

            # Weights/biases (tiny), then xT in four strictly-chained
            # column pieces. One tile + one DMA per piece (all 6 dim
            # chunks in a single transfer) keeps descriptor-issue cost off
            # the critical path, and the chain serializes arrivals in
            # consumption order so the head gets full bandwidth.
            w_sb = {}
            for name, ap in (("q", wq_ap), ("k", wk_ap), ("v", wv_ap)):
                w = singles.tile([128, NDC, D], fp16, tag=f"w{name}")
                nc.gpsimd.dma_start(w[:], ap.rearrange("(c p) j -> p c j", p=128))
                w_sb[name] = w
            b_sb = {}
            for name, ap in (("q", bq_ap), ("k", bk_ap), ("v", bv_ap)):
                t = singles.tile([D, 1], fp32, tag=f"b{name}")
                nc.gpsimd.dma_start(t[:], ap[:])
                b_sb[name] = t
            xT_sb = singles.tile([128, NDC, S], fp16, tag="xT_sb")
            xT_src = xT_ap.rearrange("(c p) j -> p c j", p=128)
            pieces = ((0, 512), (512, 1024), (1024, SK), (SK, S))
            xdma = []
            xdma.append(nc.scalar.dma_start(
                xT_sb[:, :, pieces[0][0]:pieces[0][1]],
                xT_src[:, :, pieces[0][0]:pieces[0][1]]))
            for c0, c1 in pieces[1:]:
                xdma.append(nc.sync.dma_start(
                    xT_sb[:, :, c0:c1], xT_src[:, :, c0:c1]))
            for i in range(1, 4):
                add_dep_helper(xdma[i].ins, xdma[i - 1].ins,
                               reason="xT pieces arrive in consumption order")

            QT = acts.tile([D, S], fp16, tag="QT")
            KT = acts.tile([D, SK], fp16, tag="KT")
            VT = acts.tile([D, SK], fp16, tag="VT")
            V = acts.tile([128, NKK, D], fp16, tag="V")
            Vs = acts.tile([128, NKK, D], fp16, tag="Vs")
            # Accumulated exp row-sums. S-A writes slots 0 (and 1 for the
            # kk handled as two 1024-singles); S-B writes slots 2,3;
            # zero-fill so the rsum reduce over all 4 slots is correct for
            # every kk.
            sums = acts.tile([128, NKK, 4], fp32, tag="sums")
            nc.vector.memset(sums[:], 0.0)
            rsum = acts.tile([128, NKK], fp32, tag="rsum")
            rrec = acts.tile([128, NKK], fp32, tag="rrec")
            # exp(scores) for q 0:2048 stays resident for the trailing O2
            # PV pass; q 2048:4096 rotates through a small pool consumed by
            # the PV pipelined inside S-B.
            expT_A = acts.tile([128, NKK, S // 2], fp16, tag="expT_A")

            with (
                tc.tile_pool(name="ps_sA", bufs=3, space="PSUM") as ps_sA,
                tc.tile_pool(name="ps_work", bufs=2, space="PSUM") as ps_work,
            ):
                # PE warm-up on on-chip constants while xT is in flight:
                # the HAM clock gate needs ~3.4us of sustained PE activity
                # to release. The dummies cover ~8..12us — up to the head1
                # DMA arrival — and must not overshoot: the PE queue is
                # FIFO, so excess dummies delay P1. (zeros512 serves as
                # both operands; no dependence on the gpsimd-built
                # identity, whose queue is busy early.)
                for i in range(9):
                    wp = ps_work.tile([128, 512], fp32, tag="pp",
                                      name=f"warm{i}")
                    nc.tensor.matmul(wp[:], zeros512[:, :128], zeros512[:],
                                     start=True, stop=True)

                def proj_block(wname, dst, bias, sb):
                    acc = ps_work.tile([D, 512], fp32, tag="pp",
                                       name=f"a{wname}{sb}")
                    for dc in range(NDC):
                        nc.tensor.matmul(
                            acc[:], w_sb[wname][:, dc, :],
                            xT_sb[:, dc, sb * 512:(sb + 1) * 512],
                            start=(dc == 0), stop=(dc == NDC - 1))
                    nc.vector.tensor_scalar_add(
                        dst[:, sb * 512:(sb + 1) * 512], acc[:], bias[:])

                def v_trans4(kk4):
                    pt = ps_work.tile([128, 4, D], fp16, tag="pp",
                                      name=f"pt{kk4}")
                    for k in range(4):
                        nc.tensor.transpose(
                            pt[:, k, :],
                            VT[:, (kk4 + k) * 128:(kk4 + k + 1) * 128],
                            identity[:D, :D])
                    nc.vector.tensor_copy(V[:, kk4:kk4 + 4, :], pt[:])

                # P1: exactly what the first score chunk needs; the two
                # head1-gated blocks first, the head2-gated one last.
                proj_block("q", QT, b_sb["q"], 0)
                proj_block("k", KT, b_sb["k"], 0)
                proj_block("q", QT, b_sb["q"], 1)

                # One deferred work unit per S-A chunk (~640ns spare per
                # ~1.2us chunk), ordered by DMA arrival (head2 ~13us, tail
                # ~19us, hi ~31us at ~270GB/s after the ~7.5us engine
                # preamble) and by consumer need (KT sb_i before chunk
                # kk=4i; QT sb2/3 before the qq=1 half; V/Vs and QT hi
                # before S-B). None = no unit (waiting on DMA).
                units = [
                    lambda: proj_block("k", KT, b_sb["k"], 1),
                    lambda: proj_block("v", VT, b_sb["v"], 0),
                    lambda: proj_block("v", VT, b_sb["v"], 1),
                    lambda: v_trans4(0),
                    None,
                    None,
                    lambda: proj_block("k", KT, b_sb["k"], 2),
                    lambda: proj_block("q", QT, b_sb["q"], 2),
                    lambda: proj_block("q", QT, b_sb["q"], 3),
                    lambda: proj_block("k", KT, b_sb["k"], 3),
                    lambda: proj_block("v", VT, b_sb["v"], 2),
                    lambda: proj_block("v", VT, b_sb["v"], 3),
                    lambda: v_trans4(4),
                    lambda: v_trans4(8),
                    lambda: v_trans4(12),
                    lambda: proj_block("q", QT, b_sb["q"], 4),
                    lambda: proj_block("q", QT, b_sb["q"], 5),
                    lambda: proj_block("q", QT, b_sb["q"], 6),
                    lambda: proj_block("q", QT, b_sb["q"], 7),
                ]
                ui = 0
                # S-A chunks: qq-outer so the whole first pass (q 0:1024)
                # runs off the head DMA pieces alone; 3-deep score
                # buffering lets the PE run up to two chunks ahead of the
                # scalar engine.
                for qq in range(2):
                    for kk in range(NKK):
                        ps = ps_sA.tile([128, 1024], fp32, tag="ps")
                        for j in range(2):
                            nc.tensor.matmul(
                                ps[:, j * 512:(j + 1) * 512],
                                KT[:, kk * 128:(kk + 1) * 128],
                                QT[:, qq * 1024 + j * 512:
                                   qq * 1024 + (j + 1) * 512],
                                start=True, stop=True)
                        nc.scalar.activation(
                            expT_A[:, kk, qq * 1024:(qq + 1) * 1024],
                            ps[:], mybir.ActivationFunctionType.Exp,
                            accum_out=sums[:, kk, qq:qq + 1])
                        if ui < len(units):
                            if units[ui] is not None:
                                units[ui]()
                            ui += 1
                while ui < len(units):
                    if units[ui] is not None:
                        units[ui]()
                    ui += 1

            # S-B: scores+exp for q 2048:4096 + pipelined PV(q hi half).
            def pv_matmuls(kk, po, src_tile, src_off):
                for qb in range(4):
                    nc.tensor.matmul(
                        po[qb][:], Vs[:, kk, :],
                        src_tile[:, src_off + qb * 512:
                                 src_off + (qb + 1) * 512],
                        start=(kk == 0), stop=(kk == NKK - 1))

            def drain_po(po, qb_base):
                # The scalar engine is done with exp by the time these
                # fire; split the copies across both PSUM-capable engines.
                for qb in range(4):
                    ob = outp.tile([D, 512], fp16, tag="ob")
                    (nc.scalar.copy if qb % 2 == 0
                     else nc.vector.tensor_copy)(ob[:], po[qb][:])
                    nc.sync.dma_start(
                        outT_ap[:, (qb_base + qb) * 512:
                                (qb_base + qb + 1) * 512], ob[:])

            with (
                tc.tile_pool(name="ps_sB", bufs=2, space="PSUM") as ps_sB,
                tc.tile_pool(name="ps_o1", bufs=4, space="PSUM") as ps_o1,
                tc.tile_pool(name="exphi", bufs=2) as exphi_pool,
            ):
                po1 = [ps_o1.tile([D, 512], fp32, tag="po",
                                  name=f"po1_{i}") for i in range(4)]
                prev_hi = None
                for kk in range(NKK):
                    exp_hi = exphi_pool.tile([128, S // 2], fp16,
                                             tag="exp_hi")
                    for qq in (2, 3):
                        ps = ps_sB.tile([128, 1024], fp32, tag="psb")
                        for j in range(2):
                            nc.tensor.matmul(
                                ps[:, j * 512:(j + 1) * 512],
                                KT[:, kk * 128:(kk + 1) * 128],
                                QT[:, qq * 1024 + j * 512:
                                   qq * 1024 + (j + 1) * 512],
                                start=True, stop=True)
                        nc.scalar.activation(
                            exp_hi[:, (qq - 2) * 1024:(qq - 1) * 1024],
                            ps[:], mybir.ActivationFunctionType.Exp,
                            accum_out=sums[:, kk, qq:qq + 1])
                    nc.vector.reduce_sum(rsum[:, kk:kk + 1],
                                         sums[:, kk, :],
                                         axis=mybir.AxisListType.X)
                    nc.vector.reciprocal(rrec[:, kk:kk + 1],
                                         rsum[:, kk:kk + 1])
                    nc.vector.tensor_scalar_mul(Vs[:, kk, :], V[:, kk, :],
                                                rrec[:, kk:kk + 1])
                    if kk > 0:
                        pv_matmuls(kk - 1, po1, prev_hi[:], 0)
                    prev_hi = exp_hi
                pv_matmuls(NKK - 1, po1, prev_hi[:], 0)
                drain_po(po1, 4)

            # O2: PV for q 0:2048 from the persistent expT_A, as two
            # half-sweeps so the first pair of output blocks drains (and
            # DMAs out) while the second pair is still accumulating. The
            # scalar engine is idle by now, so it shares the drain copies.
            with tc.tile_pool(name="ps_o2", bufs=4, space="PSUM") as ps_o2:
                po2 = [ps_o2.tile([D, 512], fp32, tag="po2",
                                  name=f"po2_{i}") for i in range(4)]
                for qb0 in (0, 2):
                    for kk in range(NKK):
                        for qb in (qb0, qb0 + 1):
                            nc.tensor.matmul(
                                po2[qb][:], Vs[:, kk, :],
                                expT_A[:, kk, qb * 512:(qb + 1) * 512],
                                start=(kk == 0), stop=(kk == NKK - 1))
                    for qb in (qb0, qb0 + 1):
                        ob = outp.tile([D, 512], fp16, tag="ob")
                        (nc.scalar.copy if qb % 2 == 0
                         else nc.vector.tensor_copy)(ob[:], po2[qb][:])
                        nc.sync.dma_start(
                            outT_ap[:, qb * 512:(qb + 1) * 512], ob[:])

    _dedup_ldweights(nc, mybir)
    nc.compile()
    return nc


def _dedup_ldweights(nc, mybir):
    """Drop InstLdweights that reload the weights already resident in the PE
    array (identical source AP as the previous load, with only
    non-self-loading matmuls in between). Tile's lowering emits one
    LDWEIGHTS per matmul; consecutive matmuls sharing a stationary operand
    only need the first."""
    remap = {}
    removed = 0
    for fn in nc.m.functions:
        for bb in fn.blocks:
            keep = []
            last_sig = None
            last_kept = None
            for inst in bb.instructions:
                if isinstance(inst, mybir.InstLdweights):
                    w = inst.ins[0]
                    try:
                        sig = (str(w.memref), str(w.memsetref), w.offset,
                               str(w.ap), str(w.dtype),
                               inst.perf_mode, inst.is_transpose)
                    except Exception:
                        sig = None
                    if sig is not None and last_kept is not None \
                            and sig == last_sig:
                        remap[inst.name] = last_kept.name
                        del nc.inst_map[inst.name]
                        removed += 1
                        continue
                    last_sig = sig
                    last_kept = inst
                elif isinstance(inst, mybir.InstMatmult):
                    if inst.ldweights is not False:
                        last_sig = None
                        last_kept = None
                keep.append(inst)
            if len(keep) != len(bb.instructions):
                bb.instructions[:] = keep
    if remap:
        for fn in nc.m.functions:
            for bb in fn.blocks:
                for inst in bb.instructions:
                    inst.remap_dependency_names(remap)
    return removed


def _get_module():
    if "nc" not in _CACHE:
        _CACHE["nc"] = _build_module()
    return _CACHE["nc"]


def kernel(x, Wq, bq, Wk, bk, Wv, bv, _trace=False):
    from concourse.bass_utils import run_bass_kernel_spmd

    x = np.asarray(x, dtype=np.float32)
    Wq = np.asarray(Wq, dtype=np.float32)
    bq = np.asarray(bq, dtype=np.float32)
    Wk = np.asarray(Wk, dtype=np.float32)
    bk = np.asarray(bk, dtype=np.float32)
    Wv = np.asarray(Wv, dtype=np.float32)
    bv = np.asarray(bv, dtype=np.float32)

    nc = _get_module()

    scale = np.float32(1.0 / np.sqrt(D))
    wq16 = (Wq * scale).astype(np.float16)
    wk16 = Wk.astype(np.float16)
    wv16 = Wv.astype(np.float16)
    bq_s = (bq * scale).astype(np.float32).reshape(D, 1)
    bk_s = bk.astype(np.float32).reshape(D, 1)
    bv_s = bv.astype(np.float32).reshape(D, 1)

    in_maps = []
    for c in range(N_CORES):
        b, kh = divmod(c, 2)
        xb = x[b]
        if kh:
            xb = np.concatenate([xb[SK:], xb[:SK]], axis=0)
        in_maps.append({
            "xT": np.ascontiguousarray(xb.T).astype(np.float16),
            "wq": wq16, "wk": wk16, "wv": wv16,
            "bq": bq_s, "bk": bk_s, "bv": bv_s,
        })

    res = run_bass_kernel_spmd(nc, in_maps,
                               core_ids=list(range(N_CORES)), trace=_trace)

    out = np.zeros((B, S, D), dtype=np.float32)
    for c in range(N_CORES):
        b, kh = divmod(c, 2)
        o = res.results[c]["outT"].T.astype(np.float32)  # [S, D], rolled q-order
        if kh:
            o = np.concatenate([o[SK:], o[:SK]], axis=0)
        out[b] += o
    if _trace:
        kernel.last_exec_time_ns = res.exec_time_ns
        kernel.last_result = res
    return out
